# revision 14
# baseline (speedup 1.0000x reference)
"""Trainium2 Bass kernel for nn_ContextKnowledgeEncoder.

Computation (reference semantics):
  ctx  = embed[src_tokens] * src_mask          [N,Ts,D]
  kn   = embed[know_tokens] * know_mask        [N,K,Tk,D]
  ctx_use = ctx.sum(1) / src_len               [N,D]
  kn_use  = kn.sum(2) / know_len               [N,K,D]
  ck_attn = einsum('nkd,nd->nk', kn_use, ctx_use)
  cs_enc  = kn[n, cs_ids[n]]                   [N,Tk,D]
  full_enc = concat([cs_enc, ctx], 1)          [N,Tk+Ts,D]
  full_mask = concat([cs_mask, src_mask], 1)   [N,Tk+Ts]

Strategy: data-parallel over N across 8 cores (2 examples/core), embedding
table replicated per core with one extra all-zeros row. Masked-out token ids
are remapped to the zero row on the host, so a single batched dma_gather
produces exactly the masked embedding rows. Pooling sums are computed on the
tensor engine with reciprocal-length constant weights; outputs are written
straight from the gathered tiles.
"""

import numpy as np

N, TS, K, TK, D, V = 16, 1024, 32, 128, 512, 32000
NCORES = 8
EX = N // NCORES            # examples per core
P = 128
G1 = EX * (TS + TK)         # ctx+sel gather indices per core (2304)
G1T = G1 // P               # 18 tile columns
G1E = G1T // EX             # 9 cols per example (8 ctx + 1 sel)
G2 = EX * K * TK            # knowledge gather indices per core (8192)
G2T = G2 // P               # 64 tile columns (one per (example, sentence))
GMAX = 1024                 # max indices per dma_gather (SWDGE ring capacity)
CHUNK_T = GMAX // P         # 8 tile-cols per dma_gather chunk
NCH = G2T // CHUNK_T

_prog_cache = {}


def _build_program():
    from contextlib import ExitStack

    import concourse.bacc as bacc
    import concourse.tile as tile
    from concourse import mybir

    dt = mybir.dt
    nc = bacc.Bacc(
        "TRN2",
        target_bir_lowering=False,
        debug=False,
        enable_asserts=False,
        num_devices=NCORES,
    )
    embed_t = nc.dram_tensor("embed_aug", [V + 1, D], dt.float32r, kind="ExternalInput")
    g1i_t = nc.dram_tensor("g1_idx", [P, G1 // 16], dt.int16, kind="ExternalInput")
    g2i_t = nc.dram_tensor("g2_idx", [P, G2 // 16], dt.int16, kind="ExternalInput")
    ctxr_t = nc.dram_tensor("ctx_recip", [P, EX], dt.float32, kind="ExternalInput")
    # one-hot pooling weights: knind[t][p, m] = (m == t % K) / know_len[t//K, t%K]
    knind_t = nc.dram_tensor("kn_ind", [G2T, P, K], dt.float32r, kind="ExternalInput")
    maskc_t = nc.dram_tensor("maskcat", [EX, TK + TS], dt.uint8, kind="ExternalInput")
    enc_t = nc.dram_tensor(
        "full_enc_o", [EX, TK + TS, D], dt.float32, kind="ExternalOutput"
    )
    mask_t = nc.dram_tensor(
        "full_mask_o", [EX, TK + TS], dt.uint8, kind="ExternalOutput"
    )
    ck_t = nc.dram_tensor("ck_o", [EX, K, 1], dt.float32, kind="ExternalOutput")

    with tile.TileContext(nc) as tc:
        with ExitStack() as ctx:
            consts = ctx.enter_context(tc.tile_pool(name="consts", bufs=1))
            g1p = ctx.enter_context(tc.tile_pool(name="g1p", bufs=1))
            g2p = ctx.enter_context(tc.tile_pool(name="g2p", bufs=3))
            small = ctx.enter_context(tc.tile_pool(name="small", bufs=1))
            psum = ctx.enter_context(tc.tile_pool(name="psum", bufs=1, space="PSUM"))

            g1i = consts.tile([P, G1 // 16], dt.int16, tag="g1i")
            nc.sync.dma_start(out=g1i[:], in_=g1i_t.ap())
            g2i = consts.tile([P, G2 // 16], dt.int16, tag="g2i")
            nc.sync.dma_start(out=g2i[:], in_=g2i_t.ap())
            ctxr = consts.tile([P, EX], dt.float32, tag="ctxr")
            nc.sync.dma_start(out=ctxr[:], in_=ctxr_t.ap())
            knind = consts.tile([P, G2T, K], dt.float32r, tag="knind")
            nc.sync.dma_start(
                out=knind[:], in_=knind_t.ap().rearrange("t p m -> p t m")
            )
            ones32 = consts.tile([1, K], dt.float32, tag="ones32")
            nc.vector.memset(ones32[:], 1.0)

            # full_mask is a pure byte concat of input masks: pass through.
            maskb = consts.tile([EX, TK + TS], dt.uint8, tag="maskb")
            nc.sync.dma_start(out=maskb[:], in_=maskc_t.ap())
            nc.sync.dma_start(out=mask_t.ap(), in_=maskb[:])

            psum_kn = [
                psum.tile([K, D], dt.float32, tag=f"psum_kn{e}", name=f"psum_kn{e}")
                for e in range(EX)
            ]
            psum_ctx = [
                psum.tile(
                    [1, D], dt.float32, tag=f"psum_ctx{e}", name=f"psum_ctx{e}"
                )
                for e in range(EX)
            ]

            # --- gather 1: per-example ctx tokens + selected sentence ---
            # split into <=GMAX-index calls (SWDGE descriptor ring capacity)
            g1t = g1p.tile([P, G1T, D], dt.float32, tag="g1t")
            for s in range(0, G1, GMAX):
                n = min(GMAX, G1 - s)
                nc.gpsimd.dma_gather(
                    out_ap=g1t[:, s // P : (s + n) // P, :],
                    in_ap=embed_t.ap().bitcast(dt.float32),
                    idxs_ap=g1i[:, s // 16 : (s + n) // 16],
                    num_idxs=n,
                    num_idxs_reg=n,
                    elem_size=D,
                )
            for e in range(EX):
                base = e * G1E
                # masked ctx rows -> full_enc[e, Tk:, :]
                nc.sync.dma_start(
                    out=enc_t.ap()[e, TK:, :].rearrange("(c p) d -> p c d", p=P),
                    in_=g1t[:, base : base + TS // P, :],
                )
                # selected sentence rows -> full_enc[e, :Tk, :]
                nc.sync.dma_start(
                    out=enc_t.ap()[e, 0:TK, :],
                    in_=g1t[:, base + TS // P, :],
                )
                # ctx pooling: accumulate the 8 tile-columns on DVE, then one
                # matmul for the partition reduction (scaled by 1/src_len)
                acc = small.tile(
                    [P, D], dt.float32, tag=f"acc{e}", name=f"acc{e}"
                )
                nc.vector.tensor_add(
                    out=acc[:], in0=g1t[:, base, :], in1=g1t[:, base + 1, :]
                )
                for c in range(2, TS // P):
                    nc.vector.tensor_add(
                        out=acc[:], in0=acc[:], in1=g1t[:, base + c, :]
                    )
                nc.tensor.matmul(
                    out=psum_ctx[e][:],
                    lhsT=ctxr[:, e : e + 1],
                    rhs=acc[:],
                    start=True,
                    stop=True,
                )

            # --- gather 2: knowledge tokens, one tile-col per sentence ---
            for ch in range(NCH):
                g2t = g2p.tile([P, CHUNK_T, D], dt.float32r, tag="g2t")
                i0 = ch * (GMAX // 16)
                i1 = (ch + 1) * (GMAX // 16)
                nc.gpsimd.dma_gather(
                    out_ap=g2t[:],
                    in_ap=embed_t.ap(),
                    idxs_ap=g2i[:, i0:i1],
                    num_idxs=GMAX,
                    num_idxs_reg=GMAX,
                    elem_size=D,
                )
                for t_ in range(CHUNK_T):
                    t = ch * CHUNK_T + t_
                    e, k = divmod(t, K)
                    nc.tensor.matmul(
                        out=psum_kn[e][:],
                        lhsT=knind[:, t, :],
                        rhs=g2t[:, t_, :],
                        start=(k == 0),
                        stop=(k == K - 1),
                    )

            # --- ck_attn = kn_use . ctx_use per example ---
            for e in range(EX):
                cu = small.tile([1, D], dt.float32, tag=f"cu{e}")
                nc.vector.tensor_copy(out=cu[:], in_=psum_ctx[e][:])
                bc = psum.tile([K, D], dt.float32, tag=f"bc{e}")
                nc.tensor.matmul(
                    out=bc[:],
                    lhsT=ones32[:],
                    rhs=cu[:],
                    start=True,
                    stop=True,
                )
                ku = small.tile([K, D], dt.float32, tag=f"ku{e}")
                nc.vector.tensor_copy(out=ku[:], in_=psum_kn[e][:])
                tmp = small.tile([K, D], dt.float32, tag=f"tmp{e}")
                nc.vector.tensor_mul(out=tmp[:], in0=ku[:], in1=bc[:])
                ckv = small.tile([K, 1], dt.float32, tag=f"ckv{e}")
                nc.vector.tensor_reduce(
                    out=ckv[:],
                    in_=tmp[:],
                    axis=mybir.AxisListType.X,
                    op=mybir.AluOpType.add,
                )
                nc.sync.dma_start(out=ck_t.ap()[e, :, :], in_=ckv[:])

    nc.compile()
    return nc


def get_program():
    if "nc" not in _prog_cache:
        _prog_cache["nc"] = _build_program()
    return _prog_cache["nc"]


def _wrap_idx(flat):
    """int array [n] -> int16 [128, n//16]: idx i at (partition i%16, col i//16),
    replicated 8x down the partitions for the 8 Q7 cores."""
    n = flat.shape[0]
    w = flat.reshape(n // 16, 16).T.astype(np.int16)
    return np.ascontiguousarray(np.tile(w, (8, 1)))


def _prepare_in_maps(embed, src_tokens, know_tokens, src_mask, know_mask, cs):
    embed_aug = np.concatenate(
        [np.ascontiguousarray(embed, dtype=np.float32), np.zeros((1, D), np.float32)],
        axis=0,
    )
    tok_ctx = np.where(src_mask, src_tokens, V).astype(np.int32)       # [N,Ts]
    tok_kn = np.where(know_mask, know_tokens, V).astype(np.int32)      # [N,K,Tk]
    tok_sel = tok_kn[np.arange(N), cs]                                  # [N,Tk]
    sel_mask = know_mask[np.arange(N), cs]                              # [N,Tk]
    sl = src_mask.sum(1).astype(np.float32)                             # [N]
    kl = know_mask.sum(2).astype(np.float32)                            # [N,K]
    with np.errstate(divide="ignore"):
        ctx_r = (1.0 / sl).astype(np.float32)
        kn_r = (1.0 / kl).astype(np.float32)
    maskcat = np.concatenate([sel_mask, src_mask], axis=1).astype(np.uint8)

    in_maps = []
    for c in range(NCORES):
        exs = list(range(c * EX, (c + 1) * EX))
        g1_flat = np.concatenate(
            [np.concatenate([tok_ctx[e], tok_sel[e]]) for e in exs]
        )
        g2_flat = np.concatenate([tok_kn[e].ravel() for e in exs])
        # kn_ind[t] = one-hot column (t % K) scaled by 1/know_len, replicated
        # down the 128 partitions.
        kn_ind = np.zeros((G2T, P, K), np.float32)
        for t in range(G2T):
            e, k = divmod(t, K)
            kn_ind[t, :, k] = kn_r[exs[e], k]
        in_maps.append(
            {
                "embed_aug": embed_aug,
                "g1_idx": _wrap_idx(g1_flat),
                "g2_idx": _wrap_idx(g2_flat),
                "ctx_recip": np.ascontiguousarray(
                    np.broadcast_to(ctx_r[exs][None, :], (P, EX)), np.float32
                ),
                "kn_ind": kn_ind,
                "maskcat": np.ascontiguousarray(maskcat[exs]),
            }
        )
    return in_maps


def _run(in_maps):
    import concourse.bass_utils as bass_utils

    nc = get_program()
    res = bass_utils.run_bass_kernel_spmd(
        nc, in_maps, core_ids=list(range(NCORES)), trace=False
    )
    return res.results


def kernel(**inputs):
    embed = np.asarray(inputs["embed"], dtype=np.float32)
    src_tokens = np.asarray(inputs["src_tokens"], dtype=np.int32)
    know_tokens = np.asarray(inputs["know_tokens"], dtype=np.int32)
    src_mask = np.asarray(inputs["src_mask"], dtype=bool)
    know_mask = np.asarray(inputs["know_mask"], dtype=bool)
    cs_ids = np.asarray(inputs["cs_ids"], dtype=np.int32)
    use_cs_ids = int(np.asarray(inputs["use_cs_ids"]))

    if use_cs_ids:
        cs = cs_ids
        results = _run(
            _prepare_in_maps(embed, src_tokens, know_tokens, src_mask, know_mask, cs)
        )
    else:
        # ck_attn does not depend on the selection: run once with a dummy
        # selection to get ck_attn, argmax on host, then rerun with the
        # correct selection for the gathered outputs.
        cs0 = np.zeros(N, np.int32)
        r0 = _run(
            _prepare_in_maps(embed, src_tokens, know_tokens, src_mask, know_mask, cs0)
        )
        ck = np.concatenate([r["ck_o"].reshape(EX, K) for r in r0], axis=0)
        cs = np.argmax(ck, axis=1).astype(np.int32)
        results = _run(
            _prepare_in_maps(embed, src_tokens, know_tokens, src_mask, know_mask, cs)
        )

    full_enc = np.concatenate([r["full_enc_o"] for r in results], axis=0)
    full_mask = np.concatenate([r["full_mask_o"] for r in results], axis=0).astype(bool)
    ck_attn = np.concatenate(
        [r["ck_o"].reshape(EX, K) for r in results], axis=0
    )
    return full_enc, full_mask, ck_attn
